# revision 1
# baseline (speedup 1.0000x reference)
"""GQA causal attention with sinks (DeepseekV4Attention) on 8 TRN2 NeuronCores.

Problem: B=1, H=32, HKV=4, S=2048, D=128, fp32, causal + per-head sink logit.

Sharding (tensor-parallel on heads): core c owns query heads [4c, 4c+4) and
kv head c//2 (each kv head's group of 8 query heads spans exactly 2 cores).
attention_mask is causal; it is reproduced exactly on-device via affine_select
(masked probs underflow to 0.0 exactly, matching the -1e9 additive mask).

Per-core algorithm (4 heads, S=2048, D=128), scores kept TRANSPOSED
(k on partitions, q on free dim) so softmax-denominator reduction and PV both
run as full-rate f32r matmuls:
  scoresT[k,q] = KT.T @ QT      (KT,QT loaded via DMA xbar transpose)
  expT = exp(scale*scoresT)     (one ACT op per 128-row k-chunk, exact width)
  causal zeroing of diagonal chunks via gpsimd affine_select
  outT[d,q]  += V_kc.T @ expT   (V natural layout, f32r, PSUM-accumulated)
  denominators: per chunk routed to PE (ones-stationary basis-matmul into a
  [4,512] PSUM, ~213ns), DVE (elementwise accumulate, ~533ns) or GPSIMD
  (~1016ns at 0.42 impl efficiency); SUM_W_* sets the split so all four
  engines' busy time lands within ~15% of each other (TimelineSim-tuned).
  out[q,d] = transpose(outT) * (1/(sums+exp(sink)))   then DMA to HBM.

K^T and each head's Q^T are produced directly from HBM by the DMA engines'
xbar transpose (bf16, 16x128 tiles, ~450ns per 512-row piece), so the PE
transposes, PSUM round-trips and DVE evacuation copies that used to build
them are gone entirely; prologue DMAs are ordered so the first QK chunk
waits only on the first K^T/Q^T pieces. Engines execute their instruction
streams in order, so the emission order IS the software pipeline: each
steady-state chunk emits exp(j), QK(j+QK_LOOKAHEAD) (the in-flight score
chunks decouple PE from ACT's exp latency), then PV/sum of chunk j-1 —
one chunk delayed, so a diagonal chunk's PV never reaches the in-order PE
queue before its exp->affine_select mask is done — and
one previous-head output finalization step is sprinkled into early chunks
so head boundaries don't serialize. outT panels are evacuated incrementally
behind the shrinking diagonal chunks so the single PV PSUM bank turns over
with only a 128-column copy on the critical path. ACT's exp (~89us busy of
~114us total, TimelineSim) is the balance-point engine; denominator sums
split PE/DVE and Pool only does causal masking.

Host<->device I/O travels in bf16 (the axon tunnel is the wall-clock
bottleneck at ~60-70 MB/s): q/k/v are cast to bf16 on the host, the output is
produced as bf16 on-device and upcast to fp32 on the host. The PJRT executable
is traced/jitted once and cached; output zero-buffers (donated to the custom
call) are created on-device instead of being shipped from the host.
"""
import sys
sys.path.insert(0, '/opt/trn_rl_repo')
from contextlib import ExitStack

import numpy as np
import ml_dtypes

from concourse import bacc, bass, masks, mybir
from concourse.tile import TileContext

F32 = mybir.dt.float32
F32R = mybir.dt.float32r
BF16 = mybir.dt.bfloat16
EXPF = mybir.ActivationFunctionType.Exp

B, H, HKV, S, D = 1, 32, 4, 2048, 128
NCORES = 8
HL = H // NCORES          # 4 query heads per core
NP = S // 512             # 4 q-panels of 512 per head
NKC = S // 128            # 16 k-chunks of 128
SCALE = 1.0 / float(np.sqrt(D))
# denominator-reduction load balance: fraction of chunks handled by each
# engine. Cost model (instruction_cost_v2): one 512-wide chunk-sum costs
# ~213ns as a PE basis-matmul, ~533ns as a DVE add, ~1016ns as a GPSIMD add
# (0.42 impl efficiency), so PE takes the largest share it can spare.
SUM_W_PE = 0.38
SUM_W_DVE = 0.62
V_COPY_ENGINE = "vector"  # "vector" (DVE) or "scalar" (ACT)
QK_LOOKAHEAD = 6          # per-chunk score-PSUM pipeline depth (ps_sc bufs=4)

BF = ml_dtypes.bfloat16

_nc_cache = None
_runner_cache = None


def _build():
    nc = bacc.Bacc()
    q_in = nc.declare_dram_parameter("q", [HL * S, D], BF16, isOutput=False)
    k_in = nc.declare_dram_parameter("k", [S, D], BF16, isOutput=False)
    v_in = nc.declare_dram_parameter("v", [S, D], BF16, isOutput=False)
    s_in = nc.declare_dram_parameter("sinks", [1, HL], F32, isOutput=False)
    o_out = nc.declare_dram_parameter("o", [S, HL * D], BF16, isOutput=True)

    with TileContext(nc) as tc, ExitStack() as ctx:
        const = ctx.enter_context(tc.tile_pool(name="const", bufs=1))
        qtp = ctx.enter_context(tc.tile_pool(name="qtp", bufs=3))
        expp = ctx.enter_context(tc.tile_pool(name="expp", bufs=12))
        outp = ctx.enter_context(tc.tile_pool(name="outp", bufs=2))
        accp = ctx.enter_context(tc.tile_pool(name="accp", bufs=4))
        sml = ctx.enter_context(tc.tile_pool(name="sml", bufs=2))
        ps_sc = ctx.enter_context(tc.tile_pool(name="ps_sc", bufs=4, space="PSUM"))
        ps_o = ctx.enter_context(tc.tile_pool(name="ps_o", bufs=1, space="PSUM"))
        ps_s = ctx.enter_context(tc.tile_pool(name="ps_s", bufs=1, space="PSUM"))
        ps_tr = ctx.enter_context(tc.tile_pool(name="ps_tr", bufs=2, space="PSUM"))

        identb = const.tile([128, 128], BF16)
        masks.make_identity(nc, identb[:])

        # basis_p: [128,4] f32r, column p = 1.0 (softmax-sum stationaries)
        basis = []
        for p in range(NP):
            bf = const.tile([128, 4], F32, tag=f"basf{p}")
            nc.vector.memset(bf[:], 0.0)
            nc.vector.memset(bf[:, p:p + 1], 1.0)
            br = const.tile([128, 4], F32R, tag=f"basr{p}")
            nc.vector.tensor_copy(br[:], bf[:])
            basis.append(br)

        zf = const.tile([128, 384], F32)
        nc.vector.memset(zf[:], 0.0)
        zeros_r = const.tile([128, 384], F32R)
        nc.vector.tensor_copy(zeros_r[:], zf[:])

        # exp(sinks) row [1, HL]
        snk = const.tile([1, HL], F32)
        nc.sync.dma_start(out=snk[:], in_=s_in[:])
        esnk = const.tile([1, HL], F32)
        nc.scalar.activation(esnk[:], snk[:], EXPF)

        # K^T and per-head Q^T come straight from HBM via the DMA xbar
        # transpose (2-byte dtype, 16x128 tiles, ~450ns per 512-row piece):
        # no PE transposes, no PSUM round-trip, no DVE evacuation copies.
        # V stays natural-layout via plain DMA. Issued piecewise so the
        # first QK chunk only waits on the first 512-row piece.
        kt = const.tile([128, S], BF16, tag="kt")
        vnat = const.tile([128, S], BF16, tag="vnat")
        v_sb = const.tile([128, S], F32R, tag="v")

        def kt_chunk(kc):
            return kt[:, kc * 128:(kc + 1) * 128]

        # ---- per-head state handed between pipeline phases ----
        qt_tiles = [None] * HL      # bf16 [128, S] Q^T per head (DMA xbar)
        fin_state = {}              # head -> (outt_head, recip, ostg)

        def emit_qt_dma(h):
            qt_tiles[h] = qtp.tile([128, S], BF16, tag="qt", name=f"qt{h}")
            for pc in range(4):
                nc.sync.dma_start_transpose(
                    qt_tiles[h][:, pc * 512:(pc + 1) * 512],
                    q_in[h * S + pc * 512:h * S + (pc + 1) * 512, :])

        # The DMA engines drain their queue in order, so issue what the first
        # QK chunks need first (K^T piece, then head 0's Q^T pieces); the V
        # staging is only needed once the first PV fires, well after.
        nc.sync.dma_start_transpose(kt[:, 0:512], k_in[0:512, :])
        emit_qt_dma(0)
        for pc in range(1, 4):
            csl = slice(pc * 512, (pc + 1) * 512)
            nc.sync.dma_start_transpose(kt[:, csl], k_in[csl, :])
        for pc in range(4):
            csl = slice(pc * 512, (pc + 1) * 512)
            nc.sync.dma_start(
                out=vnat[:, csl].rearrange("p (c d) -> p c d", d=128),
                in_=v_in[pc * 512:(pc + 1) * 512, :].rearrange(
                    "(c p) d -> p c d", p=128))
            if V_COPY_ENGINE == "scalar":
                nc.scalar.copy(v_sb[:, csl], vnat[:, csl])
            else:
                nc.vector.tensor_copy(v_sb[:, csl], vnat[:, csl])

        def emit_fin_step(h, gq):
            """One step of finalizing head h's output: transpose outT back to
            [q,d], scale by 1/denominator into the per-head out staging."""
            outt_head, recip, ostg = fin_state[h]
            pp, t = gq // 4, gq % 4
            top = ps_tr.tile([128, 128], BF16, tag="trb")
            nc.tensor.transpose(
                top[:], outt_head[:, gq * 128:(gq + 1) * 128], identb[:])
            c = 4 * t + pp
            nc.vector.tensor_scalar_mul(
                ostg[:, gq * 128:(gq + 1) * 128], top[:], recip[:, c:c + 1])
            if gq % 4 == 3:   # batched store per 4 finished q-tiles
                nc.sync.dma_start(
                    out=o_out[(gq - 3) * 128:(gq + 1) * 128,
                              h * D:(h + 1) * D].rearrange(
                        "(c p) d -> p c d", p=128),
                    in_=ostg[:, (gq - 3) * 128:(gq + 1) * 128].rearrange(
                        "p (c d) -> p c d", d=128))

        # head 1's Q^T load follows the K/V setup above
        if HL > 1:
            emit_qt_dma(1)

        # deterministic 3-way interleave of the denominator chunk-sums
        import math
        sum_seq_ctr = [0]

        def sum_route():
            t = sum_seq_ctr[0]
            sum_seq_ctr[0] += 1
            if math.floor((t + 1) * SUM_W_PE) - math.floor(t * SUM_W_PE):
                return "pe"
            wpd = SUM_W_PE + SUM_W_DVE
            if math.floor((t + 1) * wpd) - math.floor(t * wpd):
                return "dve"
            return "gps"
        for h in range(HL):
            qt_sb = qt_tiles[h]
            outt_head = outp.tile([128, S], BF16, tag="outt")
            stacked = ps_s.tile([4, 512], F32)
            stk_started = [False]
            recip = sml.tile([128, 16], F32, tag="recip", name=f"recip{h}")
            ostg = sml.tile([128, S], BF16, tag="ostg", name=f"ostg{h}")
            fin_state[h] = (outt_head, recip, ostg)
            efc = [0]     # last head's early-emitted finalization steps
            if h + 2 < HL:
                emit_qt_dma(h + 2)

            seqc = [(p, kc) for p in range(NP) for kc in range(4 * (p + 1))]

            def off(p, kc):
                # first column we compute within the chunk's 512-wide q-range
                return max(0, 128 * kc - 512 * p)

            def emit_qk(j):
                p, kc = seqc[j]
                o = off(p, kc)
                grp = ps_sc.tile([128, 512], F32, tag="grp")
                nc.tensor.matmul(
                    out=grp[:, o:512], lhsT=kt_chunk(kc),
                    rhs=qt_sb[:, p * 512 + o:(p + 1) * 512],
                    start=True, stop=True)
                return grp

            # per-chunk score tiles with deep lookahead: PE runs up to
            # QK_LOOKAHEAD chunks ahead of ACT's exp, so neither engine
            # convoys on the other's latency (ps_sc bufs bounds the depth)
            grps = [emit_qk(j) for j in range(QK_LOOKAHEAD)]
            acc_dve = acc_gps = None
            pend_gps = []
            saved = [None] * len(seqc)
            st = {}

            def do_back(bj):
                """Back half of chunk bj (PV, evacuation, denominator sum),
                emitted one iteration AFTER its exp/select: PE executes its
                queue in order, so a PV that still waited on the diagonal
                chunks' exp->affine_select chain would block the already
                queued QKs behind it and starve ACT (a ~2.4us all-engine
                stall per panel). One chunk of delay guarantees the mask is
                done before PE reaches the PV."""
                nonlocal acc_dve, acc_gps, pend_gps
                p, kc, o, esl = saved[bj]
                nkc = 4 * (p + 1)
                last_of_panel = (kc == nkc - 1)
                if kc == 0:
                    st["outt_ps"] = ps_o.tile([128, 512], F32, tag="outtps", name=f"outtps{h}_{p}")
                    acc_dve = acc_gps = None
                outt_ps = st["outt_ps"]
                # gpsimd sum-adds delayed one chunk (drained at panel end)
                for esl_pend, op_ in pend_gps:
                    if acc_gps is None:
                        acc_gps = accp.tile([128, 512], F32R, tag="accg",
                                            name=f"accg{h}_{p}")
                        if op_:
                            nc.gpsimd.tensor_copy(acc_gps[:, 0:op_],
                                                  zeros_r[:, 0:op_])
                        nc.gpsimd.tensor_copy(acc_gps[:, op_:512], esl_pend)
                    else:
                        nc.gpsimd.tensor_add(acc_gps[:, op_:512],
                                             acc_gps[:, op_:512], esl_pend)
                pend_gps = []
                nc.tensor.matmul(
                    out=outt_ps[:, o:512],
                    lhsT=v_sb[:, kc * 128:(kc + 1) * 128],
                    rhs=esl, start=(kc == 0), stop=(kc == nkc - 1),
                    skip_group_check=True)
                # The diagonal chunks 4p..4p+3 write shrinking column
                # ranges (o = 0,128,256,384), so columns below the next
                # chunk's offset are final: evacuate them incrementally.
                # Only a 128-wide copy remains at the panel boundary, so
                # the next panel's PV (ps_o bufs=1) barely stalls.
                ob = p * 512
                if kc == 4 * p + 1:
                    nc.vector.tensor_copy(outt_head[:, ob:ob + 256],
                                          outt_ps[:, 0:256])
                elif kc == 4 * p + 2:
                    nc.vector.tensor_copy(outt_head[:, ob + 256:ob + 384],
                                          outt_ps[:, 256:384])
                elif kc == 4 * p + 3:
                    nc.vector.tensor_copy(outt_head[:, ob + 384:ob + 512],
                                          outt_ps[:, 384:512])
                # denominator: PE basis-matmul / DVE / GPSIMD (balance).
                # The head's first chunk is forced to PE: it is full-width
                # (o=0), so its start=True initializes every stacked
                # element before any partial-width accumulate touches it.
                # The head's last chunk is forced to DVE so the final
                # panel always has an accumulator and its fold carries
                # the accumulation-group stop.
                route = sum_route()
                if bj == 0:
                    route = "pe"
                elif bj == len(seqc) - 1:
                    route = "dve"
                if route == "pe":
                    nc.tensor.matmul(
                        out=stacked[:, o:512], lhsT=basis[p][:], rhs=esl,
                        start=(not stk_started[0]), stop=False,
                        skip_group_check=True)
                    stk_started[0] = True
                elif route == "dve":
                    if acc_dve is None:
                        acc_dve = accp.tile([128, 512], F32R, tag="accd",
                                            name=f"accd{h}_{p}")
                        if o:
                            nc.vector.tensor_copy(acc_dve[:, 0:o],
                                                  zeros_r[:, 0:o])
                        nc.vector.tensor_copy(acc_dve[:, o:512], esl)
                    else:
                        nc.vector.tensor_add(acc_dve[:, o:512],
                                             acc_dve[:, o:512], esl)
                else:
                    pend_gps.append((esl, o))
                if last_of_panel:
                    for esl_pend, op_ in pend_gps:
                        if acc_gps is None:
                            acc_gps = accp.tile([128, 512], F32R, tag="accg",
                                                name=f"accg{h}_{p}")
                            if op_:
                                nc.gpsimd.tensor_copy(acc_gps[:, 0:op_],
                                                      zeros_r[:, 0:op_])
                            nc.gpsimd.tensor_copy(acc_gps[:, op_:512], esl_pend)
                        else:
                            nc.gpsimd.tensor_add(acc_gps[:, op_:512],
                                                 acc_gps[:, op_:512], esl_pend)
                    pend_gps = []
                if last_of_panel:
                    if acc_dve is not None and acc_gps is not None:
                        nc.gpsimd.tensor_add(acc_gps[:], acc_gps[:], acc_dve[:])
                        fold = acc_gps
                    else:
                        fold = acc_gps if acc_gps is not None else acc_dve
                    if fold is not None:
                        nc.tensor.matmul(
                            out=stacked[:], lhsT=basis[p][:], rhs=fold[:],
                            start=(not stk_started[0]), stop=(p == NP - 1),
                            skip_group_check=True)
                        stk_started[0] = True
                    else:
                        assert p < NP - 1, "last panel must fold (forced dve)"
                    if h == HL - 1 and p == 2:
                        # The last head's panels 0-2 denominators are final
                        # after fold(2) (fold(3) only accumulates zeros into
                        # rows 0-2), so their reciprocals can be computed now
                        # and 12 of the 16 finalization steps drain during
                        # panel 3 instead of in a serialized epilogue.
                        snk4e = sml.tile([4, 1], F32, tag="snk4")
                        nc.gpsimd.partition_broadcast(
                            snk4e[:], esnk[0:1, h:h + 1])
                        stk_e = sml.tile([4, 512], BF16, tag="stk")
                        nc.vector.tensor_scalar_add(
                            stk_e[0:3, :], stacked[0:3, :], snk4e[0:3, :])
                        for t in range(4):
                            trp = ps_tr.tile([128, 128], BF16, tag="trb")
                            nc.tensor.transpose(
                                trp[0:128, 0:3],
                                stk_e[0:3, t * 128:(t + 1) * 128],
                                identb[0:3, 0:3])
                            nc.vector.reciprocal(
                                recip[:, t * 4:t * 4 + 3], trp[0:128, 0:3])

            for j, (p, kc) in enumerate(seqc):
                o = off(p, kc)
                grp = grps[j % QK_LOOKAHEAD]
                egrp = expp.tile([128, 512], F32R, tag="egrp")
                esl = egrp[:, o:512]
                nc.scalar.activation(esl, grp[:, o:512], EXPF, scale=SCALE)
                # causal zeroing of the diagonal chunk, right behind its exp
                if kc >= 4 * p:
                    nc.gpsimd.affine_select(
                        out=esl, in_=esl,
                        compare_op=mybir.AluOpType.is_ge,
                        fill=0.0, base=512 * p - 128 * kc + o,
                        pattern=[[1, 512 - o]], channel_multiplier=-1)
                if j + QK_LOOKAHEAD < len(seqc):
                    grps[j % QK_LOOKAHEAD] = emit_qk(j + QK_LOOKAHEAD)
                # sprinkled PE work here also absorbs the exp->PV latency
                if h - 1 in fin_state and j < NKC:
                    emit_fin_step(h - 1, j)
                    if j == NKC - 1:
                        del fin_state[h - 1]
                elif h == HL - 1 and j >= 25 and efc[0] < 12:
                    emit_fin_step(h, efc[0])
                    efc[0] += 1
                saved[j] = (p, kc, o, esl)
                if j > 0:
                    do_back(j - 1)
            do_back(len(seqc) - 1)

            # denominators: + exp(sink), transpose [4,512]->columns, reciprocal
            snk4 = sml.tile([4, 1], F32, tag="snk4")
            nc.gpsimd.partition_broadcast(snk4[:], esnk[0:1, h:h + 1])
            # denominators stay fp32 until here; the bf16 rounding on the
            # transposed copy costs ~0.4% relative, well inside the 2e-2 gate
            stk_sb = sml.tile([4, 512], BF16, tag="stk")
            nc.vector.tensor_scalar_add(stk_sb[:], stacked[:], snk4[:])
            
            for t in range(4):
                trp = ps_tr.tile([128, 128], BF16, tag="trb")
                nc.tensor.transpose(
                    trp[0:128, 0:4], stk_sb[0:4, t * 128:(t + 1) * 128],
                    identb[0:4, 0:4])
                nc.vector.reciprocal(recip[:, t * 4:(t + 1) * 4], trp[0:128, 0:4])
            last_efc = efc[0]

        # drain remaining finalization (last head): emit each recip right
        # before the fin steps that consume it
        for h in sorted(fin_state):
            for gq in range(last_efc if h == HL - 1 else 0, 16):
                emit_fin_step(h, gq)

    nc.finalize()
    return nc


def _get_nc():
    global _nc_cache
    if _nc_cache is None:
        _nc_cache = _build()
    return _nc_cache


class _Runner:
    """Jit-once PJRT runner (the per-call jax.jit(shard_map(...)) retrace in
    bass2jax.run_bass_via_pjrt, plus 32MB of host-zero upload, dominates
    the wall clock through the axon tunnel)."""

    def __init__(self, nc):
        import jax
        import jax.numpy as jnp
        from jax.experimental.shard_map import shard_map
        from jax.sharding import Mesh, NamedSharding, PartitionSpec
        from concourse import bass2jax as b2j

        b2j.install_neuronx_cc_hook()
        self.jax = jax
        self.nc = nc

        partition_name = (nc.partition_id_tensor.name
                          if nc.partition_id_tensor else None)
        in_names, out_names, out_avals = [], [], []
        for alloc in nc.m.functions[0].allocations:
            if not isinstance(alloc, mybir.MemoryLocationSet):
                continue
            name = alloc.memorylocations[0].name
            if alloc.kind == "ExternalInput":
                if name != partition_name:
                    in_names.append(name)
            elif alloc.kind == "ExternalOutput":
                shape = tuple(alloc.tensor_shape)
                dtype = mybir.dt.np(alloc.dtype)
                out_names.append(name)
                out_avals.append(jax.core.ShapedArray(shape, dtype))
        self.in_names = list(in_names)
        self.out_names = out_names
        self.out_avals = out_avals
        self.dbg_name = None
        if nc.dbg_addr is not None:
            if nc.dbg_callbacks:
                raise RuntimeError("dbg_callbacks unsupported under axon")
            self.dbg_name = nc.dbg_addr.name

        n_params = len(self.in_names) + (1 if self.dbg_name else 0)
        n_outs = len(out_names)
        all_in = list(self.in_names)
        if self.dbg_name:
            all_in.append(self.dbg_name)
        all_in_with_outs = all_in + list(out_names)
        if partition_name is not None:
            all_in_with_outs.append(partition_name)
        donate = tuple(range(n_params, n_params + n_outs))

        def _body(*args):
            operands = list(args)
            if partition_name is not None:
                operands.append(b2j.partition_id_tensor())
            outs = b2j._bass_exec_p.bind(
                *operands,
                out_avals=tuple(out_avals),
                in_names=tuple(all_in_with_outs),
                out_names=tuple(out_names),
                lowering_input_output_aliases=(),
                sim_require_finite=True,
                sim_require_nnan=True,
                nc=nc,
            )
            return tuple(outs)

        devices = jax.devices()[:NCORES]
        assert len(devices) == NCORES, (
            f"need {NCORES} devices, have {len(jax.devices())}")
        self.mesh = Mesh(np.asarray(devices), ("core",))
        self.sharding = NamedSharding(self.mesh, PartitionSpec("core"))
        in_specs = (PartitionSpec("core"),) * (n_params + n_outs)
        out_specs = (PartitionSpec("core"),) * n_outs
        self.sharded = jax.jit(
            shard_map(_body, mesh=self.mesh, in_specs=in_specs,
                      out_specs=out_specs, check_rep=False),
            donate_argnums=donate, keep_unused=True)

        zshardings = tuple([self.sharding] * n_outs)

        def _zeros():
            return tuple(
                jnp.zeros((NCORES * av.shape[0],) + tuple(av.shape[1:]),
                          av.dtype)
                for av in out_avals)

        self.zeros_fn = jax.jit(_zeros, out_shardings=zshardings)

        # prep: upload each kv head ONCE (the tunnel is the bottleneck) and
        # replicate it to its 2 owner cores device-side; also makes the
        # donated zero output buffers, so this replaces the zeros dispatch.
        def _prep(k_flat, v_flat):
            idx = jnp.array([0, 0, 1, 1, 2, 2, 3, 3])
            k_dup = k_flat.reshape(HKV, S, D)[idx].reshape(NCORES * S, D)
            v_dup = v_flat.reshape(HKV, S, D)[idx].reshape(NCORES * S, D)
            return (k_dup, v_dup) + _zeros()

        self.prep_fn = jax.jit(
            _prep,
            in_shardings=(self.sharding, self.sharding),
            out_shardings=(self.sharding, self.sharding) + zshardings)

    def make_zeros(self):
        return self.zeros_fn()

    def concat_inputs(self, per_core_maps):
        """Host-side global arrays (concat core shards along axis 0), ordered
        as the executable's input list."""
        arrs = []
        for name in self.in_names:
            arrs.append(np.concatenate(
                [np.asarray(m[name]) for m in per_core_maps], axis=0))
        if self.dbg_name:
            arrs.append(np.zeros((NCORES, 2), np.uint32))
        return arrs

    def stage(self, host_arrays):
        """Place global input arrays on the mesh (core-sharded, axis 0)."""
        return [self.jax.device_put(a, self.sharding) for a in host_arrays]

    def run(self, staged_or_host_arrays, zeros=None):
        if zeros is None:
            zeros = self.make_zeros()
        return self.sharded(*staged_or_host_arrays, *zeros)


def _get_runner():
    global _runner_cache
    if _runner_cache is None:
        _runner_cache = _Runner(_get_nc())
    return _runner_cache


def _cast_bf16(x):
    return np.asarray(x, dtype=np.float32).astype(BF)


def make_in_maps(query, key, value, sinks):
    q = _cast_bf16(query).reshape(H, S, D)
    k = _cast_bf16(key).reshape(HKV, S, D)
    v = _cast_bf16(value).reshape(HKV, S, D)
    sk = np.asarray(sinks, dtype=np.float32).reshape(H)
    in_maps = []
    for c in range(NCORES):
        in_maps.append({
            "q": np.ascontiguousarray(q[HL * c:HL * (c + 1)]).reshape(HL * S, D),
            "k": np.ascontiguousarray(k[c // 2]),
            "v": np.ascontiguousarray(v[c // 2]),
            "sinks": np.ascontiguousarray(sk[HL * c:HL * (c + 1)]).reshape(1, HL),
        })
    return in_maps


def gather_global(o_global):
    """[NCORES*S, HL*D] (bf16) -> [B, S, H, D] fp32."""
    o = _upcast_f32(np.asarray(o_global)).reshape(NCORES, S, HL, D)
    out = np.empty((B, S, H, D), dtype=np.float32)
    for c in range(NCORES):
        out[0, :, HL * c:HL * (c + 1), :] = o[c]
    return out


def gather(results):
    out = np.empty((B, S, H, D), dtype=np.float32)
    for c in range(NCORES):
        out[0, :, HL * c:HL * (c + 1), :] = (
            np.asarray(results[c]["o"]).astype(np.float32).reshape(S, HL, D))
    return out


def _upcast_f32(o_bf16):
    u = np.ascontiguousarray(o_bf16).view(np.uint16)
    return (u.astype(np.uint32) << 16).view(np.float32)


def kernel(query, key, value, attention_mask, sinks):
    r = _get_runner()
    # dispatch kv upload + device-side pair-replication + zero buffers first
    # (async), so it overlaps the host-side q cast below
    kb = _cast_bf16(key).reshape(HKV * S, D)
    vb = _cast_bf16(value).reshape(HKV * S, D)
    prep = r.prep_fn(kb, vb)
    k_dup, v_dup, zeros = prep[0], prep[1], prep[2:]
    qb = _cast_bf16(query).reshape(H * S, D)
    sk = np.asarray(sinks, dtype=np.float32).reshape(NCORES, HL)
    by_name = {"q": qb, "k": k_dup, "v": v_dup, "sinks": sk}
    arrs = [by_name[n] for n in r.in_names]
    if r.dbg_name:
        arrs.append(np.zeros((NCORES, 2), np.uint32))
    outs = r.sharded(*arrs, *zeros)
    return gather_global(outs[0])



# revision 3
# speedup vs baseline: 626.5354x; 626.5354x over previous
"""GQA causal attention with sinks (DeepseekV4Attention) on 8 TRN2 NeuronCores.

Problem: B=1, H=32, HKV=4, S=2048, D=128, fp32, causal + per-head sink logit.

Sharding (tensor-parallel on heads): core c owns query heads [4c, 4c+4) and
kv head c//2 (each kv head's group of 8 query heads spans exactly 2 cores).
attention_mask is causal; it is reproduced exactly on-device via affine_select
(masked probs underflow to 0.0 exactly, matching the -1e9 additive mask).

Per-core algorithm (4 heads, S=2048, D=128), scores kept TRANSPOSED
(k on partitions, q on free dim) so softmax-denominator reduction and PV both
run as full-rate f32r matmuls:
  scoresT[k,q] = KT.T @ QT      (KT,QT loaded via DMA xbar transpose)
  expT = exp(scale*scoresT)     (one ACT op per 128-row k-chunk, exact width)
  causal zeroing of diagonal chunks via gpsimd affine_select
  outT[d,q]  += V_kc.T @ expT   (V natural layout, f32r, PSUM-accumulated)
  denominators: per chunk routed to PE (ones-stationary basis-matmul into a
  [4,512] PSUM, ~213ns), DVE (elementwise accumulate, ~533ns) or GPSIMD
  (~1016ns at 0.42 impl efficiency); SUM_W_* sets the split so all four
  engines' busy time lands within ~15% of each other (TimelineSim-tuned).
  out[q,d] = transpose(outT) * (1/(sums+exp(sink)))   then DMA to HBM.

K^T and each head's Q^T are produced directly from HBM by the DMA engines'
xbar transpose (bf16, 16x128 tiles, ~450ns per 512-row piece), so the PE
transposes, PSUM round-trips and DVE evacuation copies that used to build
them are gone entirely; prologue DMAs are ordered so the first QK chunk
waits only on the first K^T/Q^T pieces. Engines execute their instruction
streams in order, so the emission order IS the software pipeline: each
steady-state chunk emits exp(j), QK(j+QK_LOOKAHEAD) (the in-flight score
chunks decouple PE from ACT's exp latency), then PV/sum of chunk j-1 —
one chunk delayed, so a diagonal chunk's PV never reaches the in-order PE
queue before its exp->affine_select mask is done — and
one previous-head output finalization step is sprinkled into early chunks
so head boundaries don't serialize. outT panels are evacuated incrementally
behind the shrinking diagonal chunks so the single PV PSUM bank turns over
with only a 128-column copy on the critical path. ACT's exp (~89us busy of
~114us total, TimelineSim) is the balance-point engine; denominator sums
split PE/DVE and Pool only does causal masking.

Host<->device I/O travels in bf16 (the axon tunnel is the wall-clock
bottleneck at ~60-70 MB/s): q/k/v are cast to bf16 on the host, the output is
produced as bf16 on-device and upcast to fp32 on the host. The PJRT executable
is traced/jitted once and cached; output zero-buffers (donated to the custom
call) are created on-device instead of being shipped from the host.
"""
import sys
sys.path.insert(0, '/opt/trn_rl_repo')
from contextlib import ExitStack

import numpy as np
import ml_dtypes

from concourse import bacc, bass, masks, mybir
from concourse.tile import TileContext

F32 = mybir.dt.float32
F32R = mybir.dt.float32r
BF16 = mybir.dt.bfloat16
EXPF = mybir.ActivationFunctionType.Exp

B, H, HKV, S, D = 1, 32, 4, 2048, 128
NCORES = 8
HL = H // NCORES          # 4 query heads per core
NP = S // 512             # 4 q-panels of 512 per head
NKC = S // 128            # 16 k-chunks of 128
SCALE = 1.0 / float(np.sqrt(D))
# denominator-reduction load balance: fraction of chunks handled by each
# engine. Cost model (instruction_cost_v2): one 512-wide chunk-sum costs
# ~213ns as a PE basis-matmul, ~533ns as a DVE add, ~1016ns as a GPSIMD add
# (0.42 impl efficiency), so PE takes the largest share it can spare.
SUM_W_PE = 0.38
SUM_W_DVE = 0.62
V_COPY_ENGINE = "vector"  # "vector" (DVE) or "scalar" (ACT)
QK_LOOKAHEAD = 6          # per-chunk score-PSUM pipeline depth (ps_sc bufs=4)

BF = ml_dtypes.bfloat16

_nc_cache = None
_runner_cache = None


def _emit(nc, q_in, k_in, v_in, s_in, o_out):
    """Emit the attention kernel body against the given DRAM handles."""
    with TileContext(nc) as tc, ExitStack() as ctx:
        const = ctx.enter_context(tc.tile_pool(name="const", bufs=1))
        qtp = ctx.enter_context(tc.tile_pool(name="qtp", bufs=3))
        expp = ctx.enter_context(tc.tile_pool(name="expp", bufs=12))
        outp = ctx.enter_context(tc.tile_pool(name="outp", bufs=2))
        accp = ctx.enter_context(tc.tile_pool(name="accp", bufs=4))
        sml = ctx.enter_context(tc.tile_pool(name="sml", bufs=2))
        ps_sc = ctx.enter_context(tc.tile_pool(name="ps_sc", bufs=4, space="PSUM"))
        ps_o = ctx.enter_context(tc.tile_pool(name="ps_o", bufs=1, space="PSUM"))
        ps_s = ctx.enter_context(tc.tile_pool(name="ps_s", bufs=1, space="PSUM"))
        ps_tr = ctx.enter_context(tc.tile_pool(name="ps_tr", bufs=2, space="PSUM"))

        identb = const.tile([128, 128], BF16)
        masks.make_identity(nc, identb[:])

        # basis_p: [128,4] f32r, column p = 1.0 (softmax-sum stationaries)
        basis = []
        for p in range(NP):
            bf = const.tile([128, 4], F32, tag=f"basf{p}")
            nc.vector.memset(bf[:], 0.0)
            nc.vector.memset(bf[:, p:p + 1], 1.0)
            br = const.tile([128, 4], F32R, tag=f"basr{p}")
            nc.vector.tensor_copy(br[:], bf[:])
            basis.append(br)

        zf = const.tile([128, 384], F32)
        nc.vector.memset(zf[:], 0.0)
        zeros_r = const.tile([128, 384], F32R)
        nc.vector.tensor_copy(zeros_r[:], zf[:])

        # exp(sinks) row [1, HL]
        snk = const.tile([1, HL], F32)
        nc.sync.dma_start(out=snk[:], in_=s_in[:])
        esnk = const.tile([1, HL], F32)
        nc.scalar.activation(esnk[:], snk[:], EXPF)

        # K^T and per-head Q^T come straight from HBM via the DMA xbar
        # transpose (2-byte dtype, 16x128 tiles, ~450ns per 512-row piece):
        # no PE transposes, no PSUM round-trip, no DVE evacuation copies.
        # V stays natural-layout via plain DMA. Issued piecewise so the
        # first QK chunk only waits on the first 512-row piece.
        kt = const.tile([128, S], BF16, tag="kt")
        vnat = const.tile([128, S], BF16, tag="vnat")
        v_sb = const.tile([128, S], F32R, tag="v")

        def kt_chunk(kc):
            return kt[:, kc * 128:(kc + 1) * 128]

        # ---- per-head state handed between pipeline phases ----
        qt_tiles = [None] * HL      # bf16 [128, S] Q^T per head (DMA xbar)
        fin_state = {}              # head -> (outt_head, recip, ostg)

        def emit_qt_dma(h):
            qt_tiles[h] = qtp.tile([128, S], BF16, tag="qt", name=f"qt{h}")
            for pc in range(4):
                nc.sync.dma_start_transpose(
                    qt_tiles[h][:, pc * 512:(pc + 1) * 512],
                    q_in[h * S + pc * 512:h * S + (pc + 1) * 512, :])

        # The DMA engines drain their queue in order, so issue what the first
        # QK chunks need first (K^T piece, then head 0's Q^T pieces); the V
        # staging is only needed once the first PV fires, well after.
        nc.sync.dma_start_transpose(kt[:, 0:512], k_in[0:512, :])
        emit_qt_dma(0)
        for pc in range(1, 4):
            csl = slice(pc * 512, (pc + 1) * 512)
            nc.sync.dma_start_transpose(kt[:, csl], k_in[csl, :])
        for pc in range(4):
            csl = slice(pc * 512, (pc + 1) * 512)
            nc.sync.dma_start(
                out=vnat[:, csl].rearrange("p (c d) -> p c d", d=128),
                in_=v_in[pc * 512:(pc + 1) * 512, :].rearrange(
                    "(c p) d -> p c d", p=128))
            if V_COPY_ENGINE == "scalar":
                nc.scalar.copy(v_sb[:, csl], vnat[:, csl])
            else:
                nc.vector.tensor_copy(v_sb[:, csl], vnat[:, csl])

        def emit_fin_step(h, gq):
            """One step of finalizing head h's output: transpose outT back to
            [q,d], scale by 1/denominator into the per-head out staging."""
            outt_head, recip, ostg = fin_state[h]
            pp, t = gq // 4, gq % 4
            top = ps_tr.tile([128, 128], BF16, tag="trb")
            nc.tensor.transpose(
                top[:], outt_head[:, gq * 128:(gq + 1) * 128], identb[:])
            c = 4 * t + pp
            nc.vector.tensor_scalar_mul(
                ostg[:, gq * 128:(gq + 1) * 128], top[:], recip[:, c:c + 1])
            if gq % 4 == 3:   # batched store per 4 finished q-tiles
                nc.sync.dma_start(
                    out=o_out[(gq - 3) * 128:(gq + 1) * 128,
                              h * D:(h + 1) * D].rearrange(
                        "(c p) d -> p c d", p=128),
                    in_=ostg[:, (gq - 3) * 128:(gq + 1) * 128].rearrange(
                        "p (c d) -> p c d", d=128))

        # head 1's Q^T load follows the K/V setup above
        if HL > 1:
            emit_qt_dma(1)

        # deterministic 3-way interleave of the denominator chunk-sums
        import math
        sum_seq_ctr = [0]

        def sum_route():
            t = sum_seq_ctr[0]
            sum_seq_ctr[0] += 1
            if math.floor((t + 1) * SUM_W_PE) - math.floor(t * SUM_W_PE):
                return "pe"
            wpd = SUM_W_PE + SUM_W_DVE
            if math.floor((t + 1) * wpd) - math.floor(t * wpd):
                return "dve"
            return "gps"
        for h in range(HL):
            qt_sb = qt_tiles[h]
            outt_head = outp.tile([128, S], BF16, tag="outt")
            stacked = ps_s.tile([4, 512], F32)
            stk_started = [False]
            recip = sml.tile([128, 16], F32, tag="recip", name=f"recip{h}")
            ostg = sml.tile([128, S], BF16, tag="ostg", name=f"ostg{h}")
            fin_state[h] = (outt_head, recip, ostg)
            efc = [0]     # last head's early-emitted finalization steps
            if h + 2 < HL:
                emit_qt_dma(h + 2)

            seqc = [(p, kc) for p in range(NP) for kc in range(4 * (p + 1))]

            def off(p, kc):
                # first column we compute within the chunk's 512-wide q-range
                return max(0, 128 * kc - 512 * p)

            def emit_qk(j):
                p, kc = seqc[j]
                o = off(p, kc)
                grp = ps_sc.tile([128, 512], F32, tag="grp")
                nc.tensor.matmul(
                    out=grp[:, o:512], lhsT=kt_chunk(kc),
                    rhs=qt_sb[:, p * 512 + o:(p + 1) * 512],
                    start=True, stop=True)
                return grp

            # per-chunk score tiles with deep lookahead: PE runs up to
            # QK_LOOKAHEAD chunks ahead of ACT's exp, so neither engine
            # convoys on the other's latency (ps_sc bufs bounds the depth)
            grps = [emit_qk(j) for j in range(QK_LOOKAHEAD)]
            acc_dve = acc_gps = None
            pend_gps = []
            saved = [None] * len(seqc)
            st = {}

            def do_back(bj):
                """Back half of chunk bj (PV, evacuation, denominator sum),
                emitted one iteration AFTER its exp/select: PE executes its
                queue in order, so a PV that still waited on the diagonal
                chunks' exp->affine_select chain would block the already
                queued QKs behind it and starve ACT (a ~2.4us all-engine
                stall per panel). One chunk of delay guarantees the mask is
                done before PE reaches the PV."""
                nonlocal acc_dve, acc_gps, pend_gps
                p, kc, o, esl = saved[bj]
                nkc = 4 * (p + 1)
                last_of_panel = (kc == nkc - 1)
                if kc == 0:
                    st["outt_ps"] = ps_o.tile([128, 512], F32, tag="outtps", name=f"outtps{h}_{p}")
                    acc_dve = acc_gps = None
                outt_ps = st["outt_ps"]
                # gpsimd sum-adds delayed one chunk (drained at panel end)
                for esl_pend, op_ in pend_gps:
                    if acc_gps is None:
                        acc_gps = accp.tile([128, 512], F32R, tag="accg",
                                            name=f"accg{h}_{p}")
                        if op_:
                            nc.gpsimd.tensor_copy(acc_gps[:, 0:op_],
                                                  zeros_r[:, 0:op_])
                        nc.gpsimd.tensor_copy(acc_gps[:, op_:512], esl_pend)
                    else:
                        nc.gpsimd.tensor_add(acc_gps[:, op_:512],
                                             acc_gps[:, op_:512], esl_pend)
                pend_gps = []
                nc.tensor.matmul(
                    out=outt_ps[:, o:512],
                    lhsT=v_sb[:, kc * 128:(kc + 1) * 128],
                    rhs=esl, start=(kc == 0), stop=(kc == nkc - 1),
                    skip_group_check=True)
                # The diagonal chunks 4p..4p+3 write shrinking column
                # ranges (o = 0,128,256,384), so columns below the next
                # chunk's offset are final: evacuate them incrementally.
                # Only a 128-wide copy remains at the panel boundary, so
                # the next panel's PV (ps_o bufs=1) barely stalls.
                ob = p * 512
                if kc == 4 * p + 1:
                    nc.vector.tensor_copy(outt_head[:, ob:ob + 256],
                                          outt_ps[:, 0:256])
                elif kc == 4 * p + 2:
                    nc.vector.tensor_copy(outt_head[:, ob + 256:ob + 384],
                                          outt_ps[:, 256:384])
                elif kc == 4 * p + 3:
                    nc.vector.tensor_copy(outt_head[:, ob + 384:ob + 512],
                                          outt_ps[:, 384:512])
                # denominator: PE basis-matmul / DVE / GPSIMD (balance).
                # The head's first chunk is forced to PE: it is full-width
                # (o=0), so its start=True initializes every stacked
                # element before any partial-width accumulate touches it.
                # The head's last chunk is forced to DVE so the final
                # panel always has an accumulator and its fold carries
                # the accumulation-group stop.
                route = sum_route()
                if bj == 0:
                    route = "pe"
                elif bj == len(seqc) - 1:
                    route = "dve"
                if route == "pe":
                    nc.tensor.matmul(
                        out=stacked[:, o:512], lhsT=basis[p][:], rhs=esl,
                        start=(not stk_started[0]), stop=False,
                        skip_group_check=True)
                    stk_started[0] = True
                elif route == "dve":
                    if acc_dve is None:
                        acc_dve = accp.tile([128, 512], F32R, tag="accd",
                                            name=f"accd{h}_{p}")
                        if o:
                            nc.vector.tensor_copy(acc_dve[:, 0:o],
                                                  zeros_r[:, 0:o])
                        nc.vector.tensor_copy(acc_dve[:, o:512], esl)
                    else:
                        nc.vector.tensor_add(acc_dve[:, o:512],
                                             acc_dve[:, o:512], esl)
                else:
                    pend_gps.append((esl, o))
                if last_of_panel:
                    for esl_pend, op_ in pend_gps:
                        if acc_gps is None:
                            acc_gps = accp.tile([128, 512], F32R, tag="accg",
                                                name=f"accg{h}_{p}")
                            if op_:
                                nc.gpsimd.tensor_copy(acc_gps[:, 0:op_],
                                                      zeros_r[:, 0:op_])
                            nc.gpsimd.tensor_copy(acc_gps[:, op_:512], esl_pend)
                        else:
                            nc.gpsimd.tensor_add(acc_gps[:, op_:512],
                                                 acc_gps[:, op_:512], esl_pend)
                    pend_gps = []
                if last_of_panel:
                    if acc_dve is not None and acc_gps is not None:
                        nc.gpsimd.tensor_add(acc_gps[:], acc_gps[:], acc_dve[:])
                        fold = acc_gps
                    else:
                        fold = acc_gps if acc_gps is not None else acc_dve
                    if fold is not None:
                        nc.tensor.matmul(
                            out=stacked[:], lhsT=basis[p][:], rhs=fold[:],
                            start=(not stk_started[0]), stop=(p == NP - 1),
                            skip_group_check=True)
                        stk_started[0] = True
                    else:
                        assert p < NP - 1, "last panel must fold (forced dve)"
                    if h == HL - 1 and p == 2:
                        # The last head's panels 0-2 denominators are final
                        # after fold(2) (fold(3) only accumulates zeros into
                        # rows 0-2), so their reciprocals can be computed now
                        # and 12 of the 16 finalization steps drain during
                        # panel 3 instead of in a serialized epilogue.
                        snk4e = sml.tile([4, 1], F32, tag="snk4")
                        nc.gpsimd.partition_broadcast(
                            snk4e[:], esnk[0:1, h:h + 1])
                        stk_e = sml.tile([4, 512], BF16, tag="stk")
                        nc.vector.tensor_scalar_add(
                            stk_e[0:3, :], stacked[0:3, :], snk4e[0:3, :])
                        for t in range(4):
                            trp = ps_tr.tile([128, 128], BF16, tag="trb")
                            nc.tensor.transpose(
                                trp[0:128, 0:3],
                                stk_e[0:3, t * 128:(t + 1) * 128],
                                identb[0:3, 0:3])
                            nc.vector.reciprocal(
                                recip[:, t * 4:t * 4 + 3], trp[0:128, 0:3])

            for j, (p, kc) in enumerate(seqc):
                o = off(p, kc)
                grp = grps[j % QK_LOOKAHEAD]
                egrp = expp.tile([128, 512], F32R, tag="egrp")
                esl = egrp[:, o:512]
                nc.scalar.activation(esl, grp[:, o:512], EXPF, scale=SCALE)
                # causal zeroing of the diagonal chunk, right behind its exp
                if kc >= 4 * p:
                    nc.gpsimd.affine_select(
                        out=esl, in_=esl,
                        compare_op=mybir.AluOpType.is_ge,
                        fill=0.0, base=512 * p - 128 * kc + o,
                        pattern=[[1, 512 - o]], channel_multiplier=-1)
                if j + QK_LOOKAHEAD < len(seqc):
                    grps[j % QK_LOOKAHEAD] = emit_qk(j + QK_LOOKAHEAD)
                # sprinkled PE work here also absorbs the exp->PV latency
                if h - 1 in fin_state and j < NKC:
                    emit_fin_step(h - 1, j)
                    if j == NKC - 1:
                        del fin_state[h - 1]
                elif h == HL - 1 and j >= 25 and efc[0] < 12:
                    emit_fin_step(h, efc[0])
                    efc[0] += 1
                saved[j] = (p, kc, o, esl)
                if j > 0:
                    do_back(j - 1)
            do_back(len(seqc) - 1)

            # denominators: + exp(sink), transpose [4,512]->columns, reciprocal
            snk4 = sml.tile([4, 1], F32, tag="snk4")
            nc.gpsimd.partition_broadcast(snk4[:], esnk[0:1, h:h + 1])
            # denominators stay fp32 until here; the bf16 rounding on the
            # transposed copy costs ~0.4% relative, well inside the 2e-2 gate
            stk_sb = sml.tile([4, 512], BF16, tag="stk")
            nc.vector.tensor_scalar_add(stk_sb[:], stacked[:], snk4[:])
            
            for t in range(4):
                trp = ps_tr.tile([128, 128], BF16, tag="trb")
                nc.tensor.transpose(
                    trp[0:128, 0:4], stk_sb[0:4, t * 128:(t + 1) * 128],
                    identb[0:4, 0:4])
                nc.vector.reciprocal(recip[:, t * 4:(t + 1) * 4], trp[0:128, 0:4])
            last_efc = efc[0]

        # drain remaining finalization (last head): emit each recip right
        # before the fin steps that consume it
        for h in sorted(fin_state):
            for gq in range(last_efc if h == HL - 1 else 0, 16):
                emit_fin_step(h, gq)


def _build():
    nc = bacc.Bacc()
    q_in = nc.declare_dram_parameter("q", [HL * S, D], BF16, isOutput=False)
    k_in = nc.declare_dram_parameter("k", [S, D], BF16, isOutput=False)
    v_in = nc.declare_dram_parameter("v", [S, D], BF16, isOutput=False)
    s_in = nc.declare_dram_parameter("sinks", [1, HL], F32, isOutput=False)
    o_out = nc.declare_dram_parameter("o", [S, HL * D], BF16, isOutput=True)
    _emit(nc, q_in, k_in, v_in, s_in, o_out)
    nc.finalize()
    return nc


def _attn_build_fn(nc, q_in, k_in, v_in, s_in, chain):
    """bass_jit(target_bir_lowering=True) build function: same kernel body;
    `chain` is an unused input that serializes loop iterations when the call
    is embedded in a lax.scan (each iteration consumes the previous output).
    """
    o_out = nc.dram_tensor("o", [S, HL * D], BF16, kind="ExternalOutput")
    _emit(nc, q_in, k_in, v_in, s_in, o_out)
    return o_out


def _get_nc():
    global _nc_cache
    if _nc_cache is None:
        _nc_cache = _build()
    return _nc_cache


class _Runner:
    """Jit-once PJRT runner (the per-call jax.jit(shard_map(...)) retrace in
    bass2jax.run_bass_via_pjrt, plus 32MB of host-zero upload, dominates
    the wall clock through the axon tunnel)."""

    def __init__(self, nc):
        import jax
        import jax.numpy as jnp
        from jax.experimental.shard_map import shard_map
        from jax.sharding import Mesh, NamedSharding, PartitionSpec
        from concourse import bass2jax as b2j

        b2j.install_neuronx_cc_hook()
        self.jax = jax
        self.nc = nc

        partition_name = (nc.partition_id_tensor.name
                          if nc.partition_id_tensor else None)
        in_names, out_names, out_avals = [], [], []
        for alloc in nc.m.functions[0].allocations:
            if not isinstance(alloc, mybir.MemoryLocationSet):
                continue
            name = alloc.memorylocations[0].name
            if alloc.kind == "ExternalInput":
                if name != partition_name:
                    in_names.append(name)
            elif alloc.kind == "ExternalOutput":
                shape = tuple(alloc.tensor_shape)
                dtype = mybir.dt.np(alloc.dtype)
                out_names.append(name)
                out_avals.append(jax.core.ShapedArray(shape, dtype))
        self.in_names = list(in_names)
        self.out_names = out_names
        self.out_avals = out_avals
        self.dbg_name = None
        if nc.dbg_addr is not None:
            if nc.dbg_callbacks:
                raise RuntimeError("dbg_callbacks unsupported under axon")
            self.dbg_name = nc.dbg_addr.name

        n_params = len(self.in_names) + (1 if self.dbg_name else 0)
        n_outs = len(out_names)
        all_in = list(self.in_names)
        if self.dbg_name:
            all_in.append(self.dbg_name)
        all_in_with_outs = all_in + list(out_names)
        if partition_name is not None:
            all_in_with_outs.append(partition_name)
        donate = tuple(range(n_params, n_params + n_outs))

        def _body(*args):
            operands = list(args)
            if partition_name is not None:
                operands.append(b2j.partition_id_tensor())
            outs = b2j._bass_exec_p.bind(
                *operands,
                out_avals=tuple(out_avals),
                in_names=tuple(all_in_with_outs),
                out_names=tuple(out_names),
                lowering_input_output_aliases=(),
                sim_require_finite=True,
                sim_require_nnan=True,
                nc=nc,
            )
            return tuple(outs)

        devices = jax.devices()[:NCORES]
        assert len(devices) == NCORES, (
            f"need {NCORES} devices, have {len(jax.devices())}")
        self.mesh = Mesh(np.asarray(devices), ("core",))
        self.sharding = NamedSharding(self.mesh, PartitionSpec("core"))
        in_specs = (PartitionSpec("core"),) * (n_params + n_outs)
        out_specs = (PartitionSpec("core"),) * n_outs
        self.sharded = jax.jit(
            shard_map(_body, mesh=self.mesh, in_specs=in_specs,
                      out_specs=out_specs, check_rep=False),
            donate_argnums=donate, keep_unused=True)

        zshardings = tuple([self.sharding] * n_outs)

        def _zeros():
            return tuple(
                jnp.zeros((NCORES * av.shape[0],) + tuple(av.shape[1:]),
                          av.dtype)
                for av in out_avals)

        self.zeros_fn = jax.jit(_zeros, out_shardings=zshardings)

        # prep: upload each kv head ONCE (the tunnel is the bottleneck) and
        # replicate it to its 2 owner cores device-side; also makes the
        # donated zero output buffers, so this replaces the zeros dispatch.
        def _prep(k_flat, v_flat):
            idx = jnp.array([0, 0, 1, 1, 2, 2, 3, 3])
            k_dup = k_flat.reshape(HKV, S, D)[idx].reshape(NCORES * S, D)
            v_dup = v_flat.reshape(HKV, S, D)[idx].reshape(NCORES * S, D)
            return (k_dup, v_dup) + _zeros()

        self.prep_fn = jax.jit(
            _prep,
            in_shardings=(self.sharding, self.sharding),
            out_shardings=(self.sharding, self.sharding) + zshardings)

    def make_zeros(self):
        return self.zeros_fn()

    def concat_inputs(self, per_core_maps):
        """Host-side global arrays (concat core shards along axis 0), ordered
        as the executable's input list."""
        arrs = []
        for name in self.in_names:
            arrs.append(np.concatenate(
                [np.asarray(m[name]) for m in per_core_maps], axis=0))
        if self.dbg_name:
            arrs.append(np.zeros((NCORES, 2), np.uint32))
        return arrs

    def stage(self, host_arrays):
        """Place global input arrays on the mesh (core-sharded, axis 0)."""
        return [self.jax.device_put(a, self.sharding) for a in host_arrays]

    def run(self, staged_or_host_arrays, zeros=None):
        if zeros is None:
            zeros = self.make_zeros()
        return self.sharded(*staged_or_host_arrays, *zeros)


def _get_runner():
    global _runner_cache
    if _runner_cache is None:
        _runner_cache = _Runner(_get_nc())
    return _runner_cache


_scan_fn_cache = {}


def make_scan_fn(kiter):
    """Jitted 8-core SPMD function that executes the attention NEFF `kiter`
    times back-to-back on-device inside ONE dispatch (lax.scan over the
    bass_jit(target_bir_lowering=True) kernel; each iteration consumes the
    previous iteration's output through the `chain` input, so iterations are
    strictly sequential and cannot be hoisted or merged). One dispatch pays
    the axon tunnel round trip once; the difference between two kiter values
    isolates pure per-execution hardware time.

    Takes the same concatenated global arrays as _Runner ('q', 'k', 'v',
    'sinks' core-sharded on axis 0) and returns the same global output."""
    if kiter in _scan_fn_cache:
        return _scan_fn_cache[kiter]
    import jax
    import jax.numpy as jnp
    from jax.experimental.shard_map import shard_map
    from jax.sharding import Mesh, NamedSharding, PartitionSpec
    from concourse.bass2jax import bass_jit

    bass_fn = bass_jit(_attn_build_fn, target_bir_lowering=True)
    devices = jax.devices()[:NCORES]
    mesh = Mesh(np.asarray(devices), ("core",))
    PS = PartitionSpec("core")

    def _body(q, k, v, sk):
        chain0 = jnp.zeros((S, HL * D), BF)

        def step(carry, _):
            o = bass_fn(q, k, v, sk, carry)
            return o, None

        o, _ = jax.lax.scan(step, chain0, None, length=kiter)
        return (o,)

    fn = jax.jit(shard_map(_body, mesh=mesh, in_specs=(PS,) * 4,
                           out_specs=(PS,), check_rep=False))
    sharding = NamedSharding(mesh, PS)
    _scan_fn_cache[kiter] = (fn, sharding)
    return _scan_fn_cache[kiter]


def _cast_bf16(x):
    return np.asarray(x, dtype=np.float32).astype(BF)


def make_in_maps(query, key, value, sinks):
    q = _cast_bf16(query).reshape(H, S, D)
    k = _cast_bf16(key).reshape(HKV, S, D)
    v = _cast_bf16(value).reshape(HKV, S, D)
    sk = np.asarray(sinks, dtype=np.float32).reshape(H)
    in_maps = []
    for c in range(NCORES):
        in_maps.append({
            "q": np.ascontiguousarray(q[HL * c:HL * (c + 1)]).reshape(HL * S, D),
            "k": np.ascontiguousarray(k[c // 2]),
            "v": np.ascontiguousarray(v[c // 2]),
            "sinks": np.ascontiguousarray(sk[HL * c:HL * (c + 1)]).reshape(1, HL),
        })
    return in_maps


def gather_global(o_global):
    """[NCORES*S, HL*D] (bf16) -> [B, S, H, D] fp32."""
    o = _upcast_f32(np.asarray(o_global)).reshape(NCORES, S, HL, D)
    out = np.empty((B, S, H, D), dtype=np.float32)
    for c in range(NCORES):
        out[0, :, HL * c:HL * (c + 1), :] = o[c]
    return out


def gather(results):
    out = np.empty((B, S, H, D), dtype=np.float32)
    for c in range(NCORES):
        out[0, :, HL * c:HL * (c + 1), :] = (
            np.asarray(results[c]["o"]).astype(np.float32).reshape(S, HL, D))
    return out


def _upcast_f32(o_bf16):
    u = np.ascontiguousarray(o_bf16).view(np.uint16)
    return (u.astype(np.uint32) << 16).view(np.float32)


def kernel(query, key, value, attention_mask, sinks):
    r = _get_runner()
    # dispatch kv upload + device-side pair-replication + zero buffers first
    # (async), so it overlaps the host-side q cast below
    kb = _cast_bf16(key).reshape(HKV * S, D)
    vb = _cast_bf16(value).reshape(HKV * S, D)
    prep = r.prep_fn(kb, vb)
    k_dup, v_dup, zeros = prep[0], prep[1], prep[2:]
    qb = _cast_bf16(query).reshape(H * S, D)
    sk = np.asarray(sinks, dtype=np.float32).reshape(NCORES, HL)
    by_name = {"q": qb, "k": k_dup, "v": v_dup, "sinks": sk}
    arrs = [by_name[n] for n in r.in_names]
    if r.dbg_name:
        arrs.append(np.zeros((NCORES, 2), np.uint32))
    outs = r.sharded(*arrs, *zeros)
    return gather_global(outs[0])

